# revision 50
# baseline (speedup 1.0000x reference)
"""Binarized 3x3 conv + batchnorm(train) + sign, on 8 TRN2 NeuronCores.

Math: out = sign((y - mean)/sqrt(var+eps)) where y = conv(x, sign(w)) + sign(b)
and mean/var are per-channel batch stats.  Since sqrt(var+eps) > 0, the output
is exactly sign(y - mean_c): variance never needs to be computed.  The +-1
channel bias cancels in sign(y - mean), so it is dropped entirely.

Strategy (data-parallel over batch, 4 images/core):
 - 1-D Winograd F(2,3) along W (host-side input transform in fp32 during
   prep): 1.5x fewer MACs than direct conv.  d~[j] = B^T d per output-column
   pair (27 pairs), kernel g~ = G g in {+-1, +-1/2, +-3/2} (exact in e4m3),
   vertical taps stay direct (3 kh).  y_even = m0+m1+m2, y_odd = m1-m2-m3.
 - all matmuls fp8-e4m3 DoubleRow (0.5 PE cycles/output-row), 243-wide
   contiguous rhs slices (no wrap waste), 36 matmuls per output tile
   (4 j x 3 comps x 3 kh) into 4 PSUM accumulators.
 - fp32-quality via the 3-component split d~ ~= c1 + c2/64 + c3/64 with the
   /64 folded into the weights (g~, g~/64).  Measured on the reference
   inputs: 148/23.9M sign flips (rel err 5.0e-3).
 - drains combine the 4 j-accumulators into y_sb even|odd halves on
   VectorE/GpSimd, harvesting per-channel sums for the mean for free.
 - one tiny AllReduce (128x2 fp32) across the 8 cores for the global mean.
 - binarize (y + (-mean)) >= 0 on VectorE/GpSimd as fp8 {0,1} bytes, the
   last block per-tile so only one small chain trails the PE stream; host
   de-interleaves even/odd and maps to +-1 fp32.
"""

import sys

if "/opt/trn_rl_repo" not in sys.path:
    sys.path.insert(0, "/opt/trn_rl_repo")

import numpy as np
import ml_dtypes

N_CORES = 8
N_PER_CORE = 4          # images per core
CI = 256                # in channels
CO = 256                # out channels
H = W = 56
OH = OW = 54
NPIX = OH * OW          # 2916
RT = 6                  # row tiles per image (9 rows each)
RROWS = 9
NP_ = 27                # output column pairs
JP = 4                  # Winograd positions
TFREE = RROWS * NP_     # 243 outputs per tile half (even or odd)
N_TOT = N_CORES * N_PER_CORE
MEAN_SCALE = 1.0 / (N_TOT * NPIX)
C_SCALE = 64.0          # residual components stored at 64x, weights at 1/64
NT = N_PER_CORE * 2 * RT  # 48 tiles per core
# head rows 0-28, tail rows 27-55 (29 rows each); per-j plane padded to 784
# so the ci-block pair stride (4*784=3136B) stays 16B-aligned
HROWS = 29
TAIL_R0 = 27
JPAD = HROWS * NP_ + 1  # 784

FP8 = ml_dtypes.float8_e4m3


def build(nc, n_cores=N_CORES):
    """Emit the SPMD program into a bacc.Bacc instance."""
    import concourse.mybir as mybir
    from concourse import tile

    f32 = mybir.dt.float32
    fp8 = mybir.dt.float8e4
    DR = mybir.MatmulPerfMode.DoubleRow
    ACT = mybir.ActivationFunctionType
    ADD = mybir.AluOpType.add
    MUL = mybir.AluOpType.mult

    xh_d = [
        nc.dram_tensor(f"xh{c}", [N_PER_CORE, 128, 2, JP * JPAD], fp8, kind="ExternalInput")
        for c in range(3)
    ]
    xt_d = [
        nc.dram_tensor(f"xt{c}", [N_PER_CORE, 128, 2, JP * JPAD], fp8, kind="ExternalInput")
        for c in range(3)
    ]
    w1_d = nc.dram_tensor("w1", [128, 2, JP, 3, 2, 128], fp8, kind="ExternalInput")
    ws_d = nc.dram_tensor("ws", [128, 2, JP, 3, 2, 128], fp8, kind="ExternalInput")
    y_d = nc.dram_tensor("y", [N_PER_CORE, 2, 128, NPIX], mybir.dt.uint8, kind="ExternalOutput")

    with tile.TileContext(nc) as tc:
        with (
            tc.tile_pool(name="wpool", bufs=1) as wpool,
            tc.tile_pool(name="xpool", bufs=2) as xpool,
            tc.tile_pool(name="ypool", bufs=1) as ypool,
            tc.tile_pool(name="spool", bufs=1) as spool,
            tc.tile_pool(name="opool", bufs=2) as opool,
            tc.tile_pool(name="btpool", bufs=6) as btpool,
            tc.tile_pool(name="pspool", bufs=8, space="PSUM") as pspool,
            tc.tile_pool(name="drampool", bufs=2, space="DRAM") as drampool,
        ):
            w1_sb = wpool.tile([128, 2, JP, 3, 2, 128], fp8, tag="w1")
            ws_sb = wpool.tile([128, 2, JP, 3, 2, 128], fp8, tag="ws")
            y_sb = ypool.tile([128, NT * 2 * TFREE], f32)
            sums = spool.tile([128, 2 * NT], f32, tag="sums")

            # ---------------- phase 1: conv + drain (+sums) ------------------
            # HWDGE serializes all transfers; order startup by first use.
            # First 3-tile group runs component-major so the opening matmuls
            # need only w1[cb0] + comp-1 heads (j01 first).
            for n in range(N_PER_CORE):
                xh = [
                    xpool.tile([128, 2, JP * JPAD], fp8, tag=f"xh{c}", name=f"xh{c}")
                    for c in range(3)
                ]
                xt = [
                    xpool.tile([128, 2, JP * JPAD], fp8, tag=f"xt{c}", name=f"xt{c}")
                    for c in range(3)
                ]
                if n == 0:
                    nc.sync.dma_start(w1_sb[:, 0], w1_d[:, 0])
                    nc.scalar.dma_start(xh[0][:, :, 0 : 2 * JPAD], xh_d[0][n][:, :, 0 : 2 * JPAD])
                    nc.sync.dma_start(ws_sb[:, 0], ws_d[:, 0])
                    nc.scalar.dma_start(xh[1][:, :, 0 : 2 * JPAD], xh_d[1][n][:, :, 0 : 2 * JPAD])
                    nc.sync.dma_start(xh[0][:, :, 2 * JPAD :], xh_d[0][n][:, :, 2 * JPAD :])
                    nc.scalar.dma_start(xh[2][:, :, 0 : 2 * JPAD], xh_d[2][n][:, :, 0 : 2 * JPAD])
                    nc.sync.dma_start(xh[1][:, :, 2 * JPAD :], xh_d[1][n][:, :, 2 * JPAD :])
                    nc.scalar.dma_start(xh[2][:, :, 2 * JPAD :], xh_d[2][n][:, :, 2 * JPAD :])
                    nc.sync.dma_start(xt[0][:], xt_d[0][n])
                    nc.scalar.dma_start(xt[1][:], xt_d[1][n])
                    nc.sync.dma_start(xt[2][:], xt_d[2][n])
                    nc.sync.dma_start(w1_sb[:, 1], w1_d[:, 1])
                    nc.scalar.dma_start(ws_sb[:, 1], ws_d[:, 1])
                else:
                    nc.scalar.dma_start(xh[0][:], xh_d[0][n])
                    nc.scalar.dma_start(xt[0][:], xt_d[0][n])
                    nc.scalar.dma_start(xh[1][:], xh_d[1][n])
                    nc.scalar.dma_start(xt[1][:], xt_d[1][n])
                    nc.scalar.dma_start(xh[2][:], xh_d[2][n])
                    nc.scalar.dma_start(xt[2][:], xt_d[2][n])

                def emit_mm(ps_t, cb, rt, j, c, kh):
                    w_sb = w1_sb if c == 0 else ws_sb
                    row = rt * RROWS + kh
                    if rt < 3:
                        src, base = xh[c], 0
                    else:
                        src, base = xt[c], TAIL_R0
                    off = j * JPAD + (row - base) * NP_
                    nc.tensor.matmul(
                        ps_t[j][:, 0:TFREE],
                        w_sb[:, cb, j, kh],
                        src[:, :, off : off + TFREE],
                        start=(c == 0 and kh == 0),
                        stop=(c == 2 and kh == 2),
                        perf_mode=DR,
                    )

                def emit_drain(ps_t, cb, rt):
                    t = (cb * N_PER_CORE + n) * RT + rt
                    ev = y_sb[:, 2 * t * TFREE : (2 * t + 1) * TFREE]
                    od = y_sb[:, (2 * t + 1) * TFREE : (2 * t + 2) * TFREE]
                    p0 = ps_t[0][:, 0:TFREE]
                    p1 = ps_t[1][:, 0:TFREE]
                    p2 = ps_t[2][:, 0:TFREE]
                    p3 = ps_t[3][:, 0:TFREE]
                    # HW allows only ONE PSUM operand per DVE/GP op: seed
                    # each half with a ScalarE PSUM->SBUF copy, then add the
                    # remaining accumulators one at a time.
                    # even = m0+m1+m2 (GpSimd), odd = m1-m2-m3 (DVE)
                    nc.scalar.activation(ev, p0, ACT.Copy)
                    nc.scalar.activation(od, p1, ACT.Copy)
                    nc.vector.scalar_tensor_tensor(ev, p1, 1.0, ev, MUL, ADD)
                    nc.vector.scalar_tensor_tensor(
                        ev, p2, 1.0, ev, MUL, ADD,
                        accum_out=sums[:, 2 * t : 2 * t + 1],
                    )
                    nc.vector.scalar_tensor_tensor(od, p2, -1.0, od, MUL, ADD)
                    nc.vector.scalar_tensor_tensor(
                        od, p3, -1.0, od, MUL, ADD,
                        accum_out=sums[:, 2 * t + 1 : 2 * t + 2],
                    )

                if n == 0:
                    pss = [
                        [
                            pspool.tile([128, 512], f32, tag="ps", name=f"ps{i}_{j}")
                            for j in range(JP)
                        ]
                        for i in range(3)
                    ]
                    for c in range(3):
                        for j in range(JP):
                            for kh in range(3):
                                for i in range(3):
                                    emit_mm(pss[i], 0, i, j, c, kh)
                    for i in range(3):
                        emit_drain(pss[i], 0, i)
                    rest = [(0, rt) for rt in range(3, RT)] + [
                        (1, rt) for rt in range(RT)
                    ]
                else:
                    rest = [(cb, rt) for cb in range(2) for rt in range(RT)]

                for cb, rt in rest:
                    ps = [
                        pspool.tile([128, 512], f32, tag="ps", name=f"psj{j}")
                        for j in range(JP)
                    ]
                    for j in range(JP):
                        for c in range(3):
                            for kh in range(3):
                                emit_mm(ps, cb, rt, j, c, kh)
                    emit_drain(ps, cb, rt)

            # ---------------- phase 2: global mean via AllReduce ------------
            sums2 = spool.tile([128, 2], f32, tag="sums2")
            nc.vector.tensor_reduce(
                sums2[:],
                sums[:].rearrange("p (c m) -> p c m", c=2),
                axis=mybir.AxisListType.X,
                op=ADD,
            )
            neg_mean = spool.tile([128, 2], f32, tag="negmean")
            if n_cores > 1:
                cc_in = drampool.tile([128, 2], f32)
                cc_out = drampool.tile([128, 2], f32)
                nc.sync.dma_start(cc_in[:], sums2[:])
                nc.gpsimd.collective_compute(
                    "AllReduce",
                    ADD,
                    replica_groups=[list(range(n_cores))],
                    ins=[cc_in.opt()],
                    outs=[cc_out.opt()],
                )
                sums_g = spool.tile([128, 2], f32, tag="sumsg")
                nc.sync.dma_start(sums_g[:], cc_out[:])
                src_sums = sums_g
            else:
                src_sums = sums2
            nc.vector.tensor_scalar(
                neg_mean[:], src_sums[:], -MEAN_SCALE, 0.0, MUL, ADD
            )

            # ---------------- phase 3: binarize + store ---------------------
            # DVE takes 6 blocks (incl. the last, per-tile), GpSimd 2.
            IS_GE = mybir.AluOpType.is_ge
            for b in range(2 * N_PER_CORE):
                cb, n = divmod(b, N_PER_CORE)
                t0 = b * RT
                nm = neg_mean[:, cb : cb + 1]
                if b == 2 * N_PER_CORE - 1:
                    for i in range(RT):
                        t = t0 + i
                        bt = btpool.tile([128, 2 * TFREE], fp8, tag="bint")
                        nc.vector.tensor_scalar(
                            bt[:],
                            y_sb[:, 2 * t * TFREE : (2 * t + 2) * TFREE],
                            nm, 0.0, ADD, IS_GE,
                        )
                        nc.sync.dma_start(
                            y_d[n, cb][:, i * 2 * TFREE : (i + 1) * 2 * TFREE],
                            bt[:].bitcast(mybir.dt.uint8),
                        )
                else:
                    e = nc.vector
                    bin_t = opool.tile([128, RT * 2 * TFREE], fp8, tag="bin")
                    e.tensor_scalar(
                        bin_t[:],
                        y_sb[:, 2 * t0 * TFREE : 2 * (t0 + RT) * TFREE],
                        nm, 0.0, ADD, IS_GE,
                    )
                    nc.sync.dma_start(y_d[n, cb], bin_t[:].bitcast(mybir.dt.uint8))

    nc.compile()
    return nc


def prep_inputs(x, weight, bias):
    """Host-side shard + Winograd transform + fp8 split."""
    assert x.shape == (N_TOT, CI, H, W) and x.dtype == np.float32

    xs = np.ascontiguousarray(
        x.reshape(N_CORES, N_PER_CORE, 2, 128, H, W).transpose(0, 1, 3, 2, 4, 5)
    )  # [core, n, ci_f, ci_b, 56, 56]
    a = xs[..., 0:54:2]
    b = xs[..., 1:55:2]
    c = xs[..., 2:56:2]
    d = xs[..., 3:56:2]
    dt = np.stack([a - c, b + c, c - b, b - d], axis=4)  # [.., ci_b, j, 56h, 27]

    c1 = dt.astype(FP8)
    r1 = dt - c1.astype(np.float32)
    c2 = (r1 * np.float32(C_SCALE)).astype(FP8)
    r2 = r1 - c2.astype(np.float32) * np.float32(1.0 / C_SCALE)
    c3 = (r2 * np.float32(C_SCALE)).astype(FP8)

    def halves(cq):
        # [core, n, 128, 2, 4, 56, 27] -> head rows 0-28, tail rows 27-55
        hd = cq[..., 0:HROWS, :].reshape(N_CORES, N_PER_CORE, 128, 2, JP, HROWS * NP_)
        tl = cq[..., TAIL_R0:, :].reshape(N_CORES, N_PER_CORE, 128, 2, JP, HROWS * NP_)
        pad = ((0, 0),) * 5 + ((0, JPAD - HROWS * NP_),)
        shp = (N_CORES, N_PER_CORE, 128, 2, JP * JPAD)
        return np.pad(hd, pad).reshape(shp), np.pad(tl, pad).reshape(shp)

    hs, ts = zip(*(halves(q) for q in (c1, c2, c3)))

    wb = np.where(weight >= 0, np.float32(1.0), np.float32(-1.0))
    g0 = wb[:, :, :, 0]
    g1 = wb[:, :, :, 1]
    g2 = wb[:, :, :, 2]
    gt = np.stack(
        [g0, (g0 + g1 + g2) / 2, (g0 - g1 + g2) / 2, g2], axis=3
    ).astype(np.float32)  # [co, ci, kh, j]
    # [co_b, co_f, ci_b, ci_f, kh, j] -> [ci_f, co_b, j, kh, ci_b, co_f]
    g6 = gt.reshape(2, 128, 2, 128, 3, JP)
    wt = np.ascontiguousarray(g6.transpose(3, 0, 5, 4, 2, 1))
    w1 = wt.astype(FP8)
    ws = (wt * np.float32(1.0 / C_SCALE)).astype(FP8)
    assert np.all(w1.astype(np.float32) == wt)
    assert np.all(ws.astype(np.float32) * C_SCALE == wt)

    out = []
    for core in range(N_CORES):
        m = {"w1": w1, "ws": ws}
        for ci in range(3):
            m[f"xh{ci}"] = hs[ci][core]
            m[f"xt{ci}"] = ts[ci][core]
        out.append(m)
    return out


def gather(results):
    """[{y: [4,2,128,2916] fp8 {0,1}}] * 8 -> (32, 256, 54, 54) fp32 +-1.

    Per row-tile the 486 bytes are [even 9x27 | odd 9x27]; de-interleave."""
    ys = np.stack([np.asarray(r["y"]).view(FP8) for r in results]).astype(np.float32)
    ys = ys.reshape(N_CORES, N_PER_CORE, 2, 128, RT, 2, RROWS, NP_)
    out = np.empty((N_CORES, N_PER_CORE, 2, 128, RT, RROWS, OW), np.float32)
    out[..., 0::2] = ys[:, :, :, :, :, 0]
    out[..., 1::2] = ys[:, :, :, :, :, 1]
    return out.reshape(N_TOT, CO, OH, OW) * np.float32(2.0) - np.float32(1.0)


_STATE = {}


def _get_nc():
    if "nc" not in _STATE:
        import concourse.bacc as bacc

        nc = bacc.Bacc(
            "TRN2", target_bir_lowering=False, debug=False, num_devices=N_CORES
        )
        _STATE["nc"] = build(nc)
    return _STATE["nc"]


def kernel(x, weight, bias, _trace=False):
    from concourse.bass_utils import run_bass_kernel_spmd

    nc = _get_nc()
    in_maps = prep_inputs(
        np.asarray(x, np.float32),
        np.asarray(weight, np.float32),
        np.asarray(bias, np.float32),
    )
    res = run_bass_kernel_spmd(
        nc, in_maps, core_ids=list(range(N_CORES)), trace=_trace
    )
    _STATE["last_result"] = res
    return gather(res.results)


# revision 55
# speedup vs baseline: 1.0150x; 1.0150x over previous
"""Binarized 3x3 conv + batchnorm(train) + sign, on 8 TRN2 NeuronCores.

Math: out = sign((y - mean)/sqrt(var+eps)) where y = conv(x, sign(w)) + sign(b)
and mean/var are per-channel batch stats.  Since sqrt(var+eps) > 0, the output
is exactly sign(y - mean_c): variance never needs to be computed.  The +-1
channel bias cancels in sign(y - mean), so it is dropped entirely.

Strategy (data-parallel over batch, 4 images/core):
 - 1-D Winograd F(2,3) along W (host-side input transform in fp32 during
   prep): 1.5x fewer MACs than direct conv.  d~[j] = B^T d per output-column
   pair (27 pairs), kernel g~ = G g in {+-1, +-1/2, +-3/2} (exact in e4m3),
   vertical taps stay direct (3 kh).  y_even = m0+m1+m2, y_odd = m1-m2-m3.
 - all matmuls fp8-e4m3 DoubleRow (0.5 PE cycles/output-row), 243-wide
   contiguous rhs slices (no wrap waste), 36 matmuls per output tile
   (4 j x 3 comps x 3 kh) into 4 PSUM accumulators.
 - fp32-quality via the 3-component split d~ ~= c1 + c2/64 + c3/64 with the
   /64 folded into the weights (g~, g~/64).  Measured on the reference
   inputs: 148/23.9M sign flips (rel err 5.0e-3).
 - drains combine the 4 j-accumulators into y_sb even|odd halves on
   VectorE/GpSimd, harvesting per-channel sums for the mean for free.
 - one tiny AllReduce (128x2 fp32) across the 8 cores for the global mean.
 - binarize (y + (-mean)) >= 0 on VectorE/GpSimd as fp8 {0,1} bytes, the
   last block per-tile so only one small chain trails the PE stream; host
   de-interleaves even/odd and maps to +-1 fp32.
"""

import sys

if "/opt/trn_rl_repo" not in sys.path:
    sys.path.insert(0, "/opt/trn_rl_repo")

import numpy as np
import ml_dtypes

N_CORES = 8
N_PER_CORE = 4          # images per core
CI = 256                # in channels
CO = 256                # out channels
H = W = 56
OH = OW = 54
NPIX = OH * OW          # 2916
RT = 6                  # row tiles per image (9 rows each)
RROWS = 9
NP_ = 27                # output column pairs
JP = 4                  # Winograd positions
TFREE = RROWS * NP_     # 243 outputs per tile half (even or odd)
N_TOT = N_CORES * N_PER_CORE
MEAN_SCALE = 1.0 / (N_TOT * NPIX)
C_SCALE = 64.0          # residual components stored at 64x, weights at 1/64
NT = N_PER_CORE * 2 * RT  # 48 tiles per core
# head rows 0-28, tail rows 27-55 (29 rows each); per-j plane padded to 784
# so the ci-block pair stride (4*784=3136B) stays 16B-aligned
HROWS = 29
TAIL_R0 = 27
JPAD = HROWS * NP_ + 1  # 784

FP8 = ml_dtypes.float8_e4m3


def build(nc, n_cores=N_CORES):
    """Emit the SPMD program into a bacc.Bacc instance."""
    import concourse.mybir as mybir
    from concourse import tile

    f32 = mybir.dt.float32
    fp8 = mybir.dt.float8e4
    DR = mybir.MatmulPerfMode.DoubleRow
    ACT = mybir.ActivationFunctionType
    ADD = mybir.AluOpType.add
    MUL = mybir.AluOpType.mult

    xh_d = [
        nc.dram_tensor(f"xh{c}", [N_PER_CORE, 128, 2, JP * JPAD], fp8, kind="ExternalInput")
        for c in range(3)
    ]
    xt_d = [
        nc.dram_tensor(f"xt{c}", [N_PER_CORE, 128, 2, JP * JPAD], fp8, kind="ExternalInput")
        for c in range(3)
    ]
    w1_d = nc.dram_tensor("w1", [128, 2, JP, 3, 2, 128], fp8, kind="ExternalInput")
    ws_d = nc.dram_tensor("ws", [128, 2, JP, 3, 2, 128], fp8, kind="ExternalInput")
    y_d = nc.dram_tensor("y", [N_PER_CORE, 2, 128, NPIX], mybir.dt.uint8, kind="ExternalOutput")

    with tile.TileContext(nc) as tc:
        with (
            tc.tile_pool(name="wpool", bufs=1) as wpool,
            tc.tile_pool(name="xpool", bufs=2) as xpool,
            tc.tile_pool(name="ypool", bufs=1) as ypool,
            tc.tile_pool(name="spool", bufs=1) as spool,
            tc.tile_pool(name="opool", bufs=2) as opool,
            tc.tile_pool(name="btpool", bufs=6) as btpool,
            tc.tile_pool(name="pspool", bufs=8, space="PSUM") as pspool,
            tc.tile_pool(name="drampool", bufs=2, space="DRAM") as drampool,
        ):
            w1_sb = wpool.tile([128, 2, JP, 3, 2, 128], fp8, tag="w1")
            ws_sb = wpool.tile([128, 2, JP, 3, 2, 128], fp8, tag="ws")
            y_sb = ypool.tile([128, NT * 2 * TFREE], f32)
            sums = spool.tile([128, 4 * NT], f32, tag="sums")

            # ---------------- phase 1: conv + drain (+sums) ------------------
            # HWDGE serializes all transfers; order startup by first use.
            # First 3-tile group runs component-major so the opening matmuls
            # need only w1[cb0] + comp-1 heads (j01 first).
            for n in range(N_PER_CORE):
                xh = [
                    xpool.tile([128, 2, JP * JPAD], fp8, tag=f"xh{c}", name=f"xh{c}")
                    for c in range(3)
                ]
                xt = [
                    xpool.tile([128, 2, JP * JPAD], fp8, tag=f"xt{c}", name=f"xt{c}")
                    for c in range(3)
                ]
                if n == 0:
                    nc.sync.dma_start(w1_sb[:, 0], w1_d[:, 0])
                    nc.scalar.dma_start(xh[0][:, :, 0 : 2 * JPAD], xh_d[0][n][:, :, 0 : 2 * JPAD])
                    nc.sync.dma_start(ws_sb[:, 0], ws_d[:, 0])
                    nc.scalar.dma_start(xh[1][:, :, 0 : 2 * JPAD], xh_d[1][n][:, :, 0 : 2 * JPAD])
                    nc.sync.dma_start(xh[0][:, :, 2 * JPAD :], xh_d[0][n][:, :, 2 * JPAD :])
                    nc.scalar.dma_start(xh[2][:, :, 0 : 2 * JPAD], xh_d[2][n][:, :, 0 : 2 * JPAD])
                    nc.sync.dma_start(xh[1][:, :, 2 * JPAD :], xh_d[1][n][:, :, 2 * JPAD :])
                    nc.scalar.dma_start(xh[2][:, :, 2 * JPAD :], xh_d[2][n][:, :, 2 * JPAD :])
                    nc.sync.dma_start(xt[0][:], xt_d[0][n])
                    nc.scalar.dma_start(xt[1][:], xt_d[1][n])
                    nc.sync.dma_start(xt[2][:], xt_d[2][n])
                    nc.sync.dma_start(w1_sb[:, 1], w1_d[:, 1])
                    nc.scalar.dma_start(ws_sb[:, 1], ws_d[:, 1])
                else:
                    nc.sync.dma_start(xh[0][:], xh_d[0][n])
                    nc.sync.dma_start(xt[0][:], xt_d[0][n])
                    nc.sync.dma_start(xh[1][:], xh_d[1][n])
                    nc.sync.dma_start(xt[1][:], xt_d[1][n])
                    nc.sync.dma_start(xh[2][:], xh_d[2][n])
                    nc.sync.dma_start(xt[2][:], xt_d[2][n])

                def emit_mm(ps_t, cb, rt, j, c, kh):
                    w_sb = w1_sb if c == 0 else ws_sb
                    row = rt * RROWS + kh
                    if rt < 3:
                        src, base = xh[c], 0
                    else:
                        src, base = xt[c], TAIL_R0
                    off = j * JPAD + (row - base) * NP_
                    nc.tensor.matmul(
                        ps_t[j][:, 0:TFREE],
                        w_sb[:, cb, j, kh],
                        src[:, :, off : off + TFREE],
                        start=(c == 0 and kh == 0),
                        stop=(c == 2 and kh == 2),
                        perf_mode=DR,
                    )

                def emit_drain(ps_t, cb, rt):
                    t = (cb * N_PER_CORE + n) * RT + rt
                    ev = y_sb[:, 2 * t * TFREE : (2 * t + 1) * TFREE]
                    od = y_sb[:, (2 * t + 1) * TFREE : (2 * t + 2) * TFREE]
                    p0 = ps_t[0][:, 0:TFREE]
                    p1 = ps_t[1][:, 0:TFREE]
                    p2 = ps_t[2][:, 0:TFREE]
                    p3 = ps_t[3][:, 0:TFREE]
                    # HW: only ONE PSUM operand per DVE op, and GpSimd
                    # cannot read PSUM at all.  ScalarE copies j0/j1/j2 out
                    # (harvesting their sums A,B,C), GpSimd adds the even
                    # half in SBUF, DVE adds the odd half (PSUM j3 last,
                    # harvesting O).  Sigma-y per tile = A+B+C+O.
                    s2 = btpool.tile([128, TFREE], f32, tag="s2", name="s2")
                    nc.scalar.activation(
                        ev, p0, ACT.Copy, accum_out=sums[:, 4 * t : 4 * t + 1]
                    )
                    nc.scalar.activation(
                        od, p1, ACT.Copy, accum_out=sums[:, 4 * t + 1 : 4 * t + 2]
                    )
                    nc.scalar.activation(
                        s2[:], p2, ACT.Copy, accum_out=sums[:, 4 * t + 2 : 4 * t + 3]
                    )
                    nc.vector.scalar_tensor_tensor(ev, od, 1.0, ev, MUL, ADD)
                    nc.vector.scalar_tensor_tensor(ev, s2[:], 1.0, ev, MUL, ADD)
                    nc.vector.scalar_tensor_tensor(od, s2[:], -1.0, od, MUL, ADD)
                    nc.vector.scalar_tensor_tensor(
                        od, p3, -1.0, od, MUL, ADD,
                        accum_out=sums[:, 4 * t + 3 : 4 * t + 4],
                    )

                if n == 0:
                    pss = [
                        [
                            pspool.tile([128, 512], f32, tag="ps", name=f"ps{i}_{j}")
                            for j in range(JP)
                        ]
                        for i in range(3)
                    ]
                    for c in range(3):
                        for j in range(JP):
                            for kh in range(3):
                                for i in range(3):
                                    emit_mm(pss[i], 0, i, j, c, kh)
                    for i in range(3):
                        emit_drain(pss[i], 0, i)
                    rest = [(0, rt) for rt in range(3, RT)] + [
                        (1, rt) for rt in range(RT)
                    ]
                else:
                    rest = [(cb, rt) for cb in range(2) for rt in range(RT)]

                for cb, rt in rest:
                    ps = [
                        pspool.tile([128, 512], f32, tag="ps", name=f"psj{j}")
                        for j in range(JP)
                    ]
                    for j in range(JP):
                        for c in range(3):
                            for kh in range(3):
                                emit_mm(ps, cb, rt, j, c, kh)
                    emit_drain(ps, cb, rt)

            # ---------------- phase 2: global mean via AllReduce ------------
            sums2 = spool.tile([128, 2], f32, tag="sums2")
            nc.vector.tensor_reduce(
                sums2[:],
                sums[:].rearrange("p (c m) -> p c m", c=2),
                axis=mybir.AxisListType.X,
                op=ADD,
            )
            neg_mean = spool.tile([128, 2], f32, tag="negmean")
            if n_cores > 1:
                cc_in = drampool.tile([128, 2], f32)
                cc_out = drampool.tile([128, 2], f32)
                nc.sync.dma_start(cc_in[:], sums2[:])
                nc.gpsimd.collective_compute(
                    "AllReduce",
                    ADD,
                    replica_groups=[list(range(n_cores))],
                    ins=[cc_in.opt()],
                    outs=[cc_out.opt()],
                )
                sums_g = spool.tile([128, 2], f32, tag="sumsg")
                nc.sync.dma_start(sums_g[:], cc_out[:])
                src_sums = sums_g
            else:
                src_sums = sums2
            nc.vector.tensor_scalar(
                neg_mean[:], src_sums[:], -MEAN_SCALE, 0.0, MUL, ADD
            )

            # ---------------- phase 3: binarize + store ---------------------
            # DVE takes 6 blocks (incl. the last, per-tile), GpSimd 2.
            IS_GE = mybir.AluOpType.is_ge
            for b in range(2 * N_PER_CORE):
                cb, n = divmod(b, N_PER_CORE)
                t0 = b * RT
                nm = neg_mean[:, cb : cb + 1]
                if b == 2 * N_PER_CORE - 1:
                    for i in range(RT):
                        t = t0 + i
                        bt = btpool.tile([128, 2 * TFREE], fp8, tag="bint")
                        nc.vector.tensor_scalar(
                            bt[:],
                            y_sb[:, 2 * t * TFREE : (2 * t + 2) * TFREE],
                            nm, 0.0, ADD, IS_GE,
                        )
                        nc.sync.dma_start(
                            y_d[n, cb][:, i * 2 * TFREE : (i + 1) * 2 * TFREE],
                            bt[:].bitcast(mybir.dt.uint8),
                        )
                else:
                    e = nc.gpsimd if b in (0, 2) else nc.vector
                    bin_t = opool.tile([128, RT * 2 * TFREE], fp8, tag="bin")
                    e.tensor_scalar(
                        bin_t[:],
                        y_sb[:, 2 * t0 * TFREE : 2 * (t0 + RT) * TFREE],
                        nm, 0.0, ADD, IS_GE,
                    )
                    nc.sync.dma_start(y_d[n, cb], bin_t[:].bitcast(mybir.dt.uint8))

    nc.compile()
    return nc


def prep_inputs(x, weight, bias):
    """Host-side shard + Winograd transform + fp8 split."""
    assert x.shape == (N_TOT, CI, H, W) and x.dtype == np.float32

    xs = np.ascontiguousarray(
        x.reshape(N_CORES, N_PER_CORE, 2, 128, H, W).transpose(0, 1, 3, 2, 4, 5)
    )  # [core, n, ci_f, ci_b, 56, 56]
    a = xs[..., 0:54:2]
    b = xs[..., 1:55:2]
    c = xs[..., 2:56:2]
    d = xs[..., 3:56:2]
    dt = np.stack([a - c, b + c, c - b, b - d], axis=4)  # [.., ci_b, j, 56h, 27]

    c1 = dt.astype(FP8)
    r1 = dt - c1.astype(np.float32)
    c2 = (r1 * np.float32(C_SCALE)).astype(FP8)
    r2 = r1 - c2.astype(np.float32) * np.float32(1.0 / C_SCALE)
    c3 = (r2 * np.float32(C_SCALE)).astype(FP8)

    def halves(cq):
        # [core, n, 128, 2, 4, 56, 27] -> head rows 0-28, tail rows 27-55
        hd = cq[..., 0:HROWS, :].reshape(N_CORES, N_PER_CORE, 128, 2, JP, HROWS * NP_)
        tl = cq[..., TAIL_R0:, :].reshape(N_CORES, N_PER_CORE, 128, 2, JP, HROWS * NP_)
        pad = ((0, 0),) * 5 + ((0, JPAD - HROWS * NP_),)
        shp = (N_CORES, N_PER_CORE, 128, 2, JP * JPAD)
        return np.pad(hd, pad).reshape(shp), np.pad(tl, pad).reshape(shp)

    hs, ts = zip(*(halves(q) for q in (c1, c2, c3)))

    wb = np.where(weight >= 0, np.float32(1.0), np.float32(-1.0))
    g0 = wb[:, :, :, 0]
    g1 = wb[:, :, :, 1]
    g2 = wb[:, :, :, 2]
    gt = np.stack(
        [g0, (g0 + g1 + g2) / 2, (g0 - g1 + g2) / 2, g2], axis=3
    ).astype(np.float32)  # [co, ci, kh, j]
    # [co_b, co_f, ci_b, ci_f, kh, j] -> [ci_f, co_b, j, kh, ci_b, co_f]
    g6 = gt.reshape(2, 128, 2, 128, 3, JP)
    wt = np.ascontiguousarray(g6.transpose(3, 0, 5, 4, 2, 1))
    w1 = wt.astype(FP8)
    ws = (wt * np.float32(1.0 / C_SCALE)).astype(FP8)
    assert np.all(w1.astype(np.float32) == wt)
    assert np.all(ws.astype(np.float32) * C_SCALE == wt)

    out = []
    for core in range(N_CORES):
        m = {"w1": w1, "ws": ws}
        for ci in range(3):
            m[f"xh{ci}"] = hs[ci][core]
            m[f"xt{ci}"] = ts[ci][core]
        out.append(m)
    return out


def gather(results):
    """[{y: [4,2,128,2916] fp8 {0,1}}] * 8 -> (32, 256, 54, 54) fp32 +-1.

    Per row-tile the 486 bytes are [even 9x27 | odd 9x27]; de-interleave."""
    ys = np.stack([np.asarray(r["y"]).view(FP8) for r in results]).astype(np.float32)
    ys = ys.reshape(N_CORES, N_PER_CORE, 2, 128, RT, 2, RROWS, NP_)
    out = np.empty((N_CORES, N_PER_CORE, 2, 128, RT, RROWS, OW), np.float32)
    out[..., 0::2] = ys[:, :, :, :, :, 0]
    out[..., 1::2] = ys[:, :, :, :, :, 1]
    return out.reshape(N_TOT, CO, OH, OW) * np.float32(2.0) - np.float32(1.0)


_STATE = {}


def _get_nc():
    if "nc" not in _STATE:
        import concourse.bacc as bacc

        nc = bacc.Bacc(
            "TRN2", target_bir_lowering=False, debug=False, num_devices=N_CORES
        )
        _STATE["nc"] = build(nc)
    return _STATE["nc"]


def kernel(x, weight, bias, _trace=False):
    from concourse.bass_utils import run_bass_kernel_spmd

    nc = _get_nc()
    in_maps = prep_inputs(
        np.asarray(x, np.float32),
        np.asarray(weight, np.float32),
        np.asarray(bias, np.float32),
    )
    res = run_bass_kernel_spmd(
        nc, in_maps, core_ids=list(range(N_CORES)), trace=_trace
    )
    _STATE["last_result"] = res
    return gather(res.results)


# revision 56
# speedup vs baseline: 1.0189x; 1.0038x over previous
"""Binarized 3x3 conv + batchnorm(train) + sign, on 8 TRN2 NeuronCores.

Math: out = sign((y - mean)/sqrt(var+eps)) where y = conv(x, sign(w)) + sign(b)
and mean/var are per-channel batch stats.  Since sqrt(var+eps) > 0, the output
is exactly sign(y - mean_c): variance never needs to be computed.  The +-1
channel bias cancels in sign(y - mean), so it is dropped entirely.

Strategy (data-parallel over batch, 4 images/core):
 - 1-D Winograd F(2,3) along W (host-side input transform in fp32 during
   prep): 1.5x fewer MACs than direct conv.  d~[j] = B^T d per output-column
   pair (27 pairs), kernel g~ = G g in {+-1, +-1/2, +-3/2} (exact in e4m3),
   vertical taps stay direct (3 kh).  y_even = m0+m1+m2, y_odd = m1-m2-m3.
 - all matmuls fp8-e4m3 DoubleRow (0.5 PE cycles/output-row), 243-wide
   contiguous rhs slices (no wrap waste), 36 matmuls per output tile
   (4 j x 3 comps x 3 kh) into 4 PSUM accumulators.
 - fp32-quality via the 3-component split d~ ~= c1 + c2/64 + c3/64 with the
   /64 folded into the weights (g~, g~/64).  Measured on the reference
   inputs: 148/23.9M sign flips (rel err 5.0e-3).
 - drains combine the 4 j-accumulators into y_sb even|odd halves on
   VectorE/GpSimd, harvesting per-channel sums for the mean for free.
 - one tiny AllReduce (128x2 fp32) across the 8 cores for the global mean.
 - binarize (y + (-mean)) >= 0 on VectorE/GpSimd as fp8 {0,1} bytes, the
   last block per-tile so only one small chain trails the PE stream; host
   de-interleaves even/odd and maps to +-1 fp32.
"""

import sys

if "/opt/trn_rl_repo" not in sys.path:
    sys.path.insert(0, "/opt/trn_rl_repo")

import numpy as np
import ml_dtypes

N_CORES = 8
N_PER_CORE = 4          # images per core
CI = 256                # in channels
CO = 256                # out channels
H = W = 56
OH = OW = 54
NPIX = OH * OW          # 2916
RT = 6                  # row tiles per image (9 rows each)
RROWS = 9
NP_ = 27                # output column pairs
JP = 4                  # Winograd positions
TFREE = RROWS * NP_     # 243 outputs per tile half (even or odd)
N_TOT = N_CORES * N_PER_CORE
MEAN_SCALE = 1.0 / (N_TOT * NPIX)
C_SCALE = 64.0          # residual components stored at 64x, weights at 1/64
NT = N_PER_CORE * 2 * RT  # 48 tiles per core
# head rows 0-28, tail rows 27-55 (29 rows each); per-j plane padded to 784
# so the ci-block pair stride (4*784=3136B) stays 16B-aligned
HROWS = 29
TAIL_R0 = 27
JPAD = HROWS * NP_ + 1  # 784

FP8 = ml_dtypes.float8_e4m3


def build(nc, n_cores=N_CORES):
    """Emit the SPMD program into a bacc.Bacc instance."""
    import concourse.mybir as mybir
    from concourse import tile

    f32 = mybir.dt.float32
    fp8 = mybir.dt.float8e4
    DR = mybir.MatmulPerfMode.DoubleRow
    ACT = mybir.ActivationFunctionType
    ADD = mybir.AluOpType.add
    MUL = mybir.AluOpType.mult

    xh_d = [
        nc.dram_tensor(f"xh{c}", [N_PER_CORE, 128, 2, JP * JPAD], fp8, kind="ExternalInput")
        for c in range(3)
    ]
    xt_d = [
        nc.dram_tensor(f"xt{c}", [N_PER_CORE, 128, 2, JP * JPAD], fp8, kind="ExternalInput")
        for c in range(3)
    ]
    w1_d = nc.dram_tensor("w1", [128, 2, JP, 3, 2, 128], fp8, kind="ExternalInput")
    ws_d = nc.dram_tensor("ws", [128, 2, JP, 3, 2, 128], fp8, kind="ExternalInput")
    y_d = nc.dram_tensor("y", [N_PER_CORE, 2, 128, NPIX], mybir.dt.uint8, kind="ExternalOutput")

    with tile.TileContext(nc) as tc:
        with (
            tc.tile_pool(name="wpool", bufs=1) as wpool,
            tc.tile_pool(name="xpool", bufs=2) as xpool,
            tc.tile_pool(name="ypool", bufs=1) as ypool,
            tc.tile_pool(name="spool", bufs=1) as spool,
            tc.tile_pool(name="opool", bufs=2) as opool,
            tc.tile_pool(name="btpool", bufs=6) as btpool,
            tc.tile_pool(name="pspool", bufs=8, space="PSUM") as pspool,
            tc.tile_pool(name="drampool", bufs=2, space="DRAM") as drampool,
        ):
            w1_sb = wpool.tile([128, 2, JP, 3, 2, 128], fp8, tag="w1")
            ws_sb = wpool.tile([128, 2, JP, 3, 2, 128], fp8, tag="ws")
            y_sb = ypool.tile([128, NT * 2 * TFREE], f32)
            sums = spool.tile([128, 4 * NT], f32, tag="sums")

            # ---------------- phase 1: conv + drain (+sums) ------------------
            # HWDGE serializes all transfers; order startup by first use.
            # First 3-tile group runs component-major so the opening matmuls
            # need only w1[cb0] + comp-1 heads (j01 first).
            for n in range(N_PER_CORE):
                xh = [
                    xpool.tile([128, 2, JP * JPAD], fp8, tag=f"xh{c}", name=f"xh{c}")
                    for c in range(3)
                ]
                xt = [
                    xpool.tile([128, 2, JP * JPAD], fp8, tag=f"xt{c}", name=f"xt{c}")
                    for c in range(3)
                ]
                if n == 0:
                    nc.sync.dma_start(w1_sb[:, 0], w1_d[:, 0])
                    nc.scalar.dma_start(xh[0][:, :, 0 : 2 * JPAD], xh_d[0][n][:, :, 0 : 2 * JPAD])
                    nc.sync.dma_start(ws_sb[:, 0], ws_d[:, 0])
                    nc.scalar.dma_start(xh[1][:, :, 0 : 2 * JPAD], xh_d[1][n][:, :, 0 : 2 * JPAD])
                    nc.sync.dma_start(xh[0][:, :, 2 * JPAD :], xh_d[0][n][:, :, 2 * JPAD :])
                    nc.scalar.dma_start(xh[2][:, :, 0 : 2 * JPAD], xh_d[2][n][:, :, 0 : 2 * JPAD])
                    nc.sync.dma_start(xh[1][:, :, 2 * JPAD :], xh_d[1][n][:, :, 2 * JPAD :])
                    nc.scalar.dma_start(xt[0][:], xt_d[0][n])
                    nc.sync.dma_start(xh[2][:, :, 2 * JPAD :], xh_d[2][n][:, :, 2 * JPAD :])
                    nc.scalar.dma_start(xt[1][:], xt_d[1][n])
                    nc.sync.dma_start(xt[2][:], xt_d[2][n])
                    nc.sync.dma_start(w1_sb[:, 1], w1_d[:, 1])
                    nc.scalar.dma_start(ws_sb[:, 1], ws_d[:, 1])
                else:
                    nc.sync.dma_start(xh[0][:], xh_d[0][n])
                    nc.sync.dma_start(xt[0][:], xt_d[0][n])
                    nc.sync.dma_start(xh[1][:], xh_d[1][n])
                    nc.sync.dma_start(xt[1][:], xt_d[1][n])
                    nc.sync.dma_start(xh[2][:], xh_d[2][n])
                    nc.sync.dma_start(xt[2][:], xt_d[2][n])

                def emit_mm(ps_t, cb, rt, j, c, kh):
                    w_sb = w1_sb if c == 0 else ws_sb
                    row = rt * RROWS + kh
                    if rt < 3:
                        src, base = xh[c], 0
                    else:
                        src, base = xt[c], TAIL_R0
                    off = j * JPAD + (row - base) * NP_
                    nc.tensor.matmul(
                        ps_t[j][:, 0:TFREE],
                        w_sb[:, cb, j, kh],
                        src[:, :, off : off + TFREE],
                        start=(c == 0 and kh == 0),
                        stop=(c == 2 and kh == 2),
                        perf_mode=DR,
                    )

                def emit_drain(ps_t, cb, rt):
                    t = (cb * N_PER_CORE + n) * RT + rt
                    ev = y_sb[:, 2 * t * TFREE : (2 * t + 1) * TFREE]
                    od = y_sb[:, (2 * t + 1) * TFREE : (2 * t + 2) * TFREE]
                    p0 = ps_t[0][:, 0:TFREE]
                    p1 = ps_t[1][:, 0:TFREE]
                    p2 = ps_t[2][:, 0:TFREE]
                    p3 = ps_t[3][:, 0:TFREE]
                    # HW: only ONE PSUM operand per DVE op, and GpSimd
                    # cannot read PSUM at all.  ScalarE copies j0/j1/j2 out
                    # (harvesting their sums A,B,C), GpSimd adds the even
                    # half in SBUF, DVE adds the odd half (PSUM j3 last,
                    # harvesting O).  Sigma-y per tile = A+B+C+O.
                    s2 = btpool.tile([128, TFREE], f32, tag="s2", name="s2")
                    nc.scalar.activation(
                        ev, p0, ACT.Copy, accum_out=sums[:, 4 * t : 4 * t + 1]
                    )
                    nc.scalar.activation(
                        od, p1, ACT.Copy, accum_out=sums[:, 4 * t + 1 : 4 * t + 2]
                    )
                    nc.scalar.activation(
                        s2[:], p2, ACT.Copy, accum_out=sums[:, 4 * t + 2 : 4 * t + 3]
                    )
                    nc.vector.scalar_tensor_tensor(ev, od, 1.0, ev, MUL, ADD)
                    nc.vector.scalar_tensor_tensor(ev, s2[:], 1.0, ev, MUL, ADD)
                    nc.vector.scalar_tensor_tensor(od, s2[:], -1.0, od, MUL, ADD)
                    nc.vector.scalar_tensor_tensor(
                        od, p3, -1.0, od, MUL, ADD,
                        accum_out=sums[:, 4 * t + 3 : 4 * t + 4],
                    )

                if n == 0:
                    # both cb0 tile groups run component-major so each comp's
                    # transfers stream in just ahead of their first use
                    for g in range(2):
                        pss = [
                            [
                                pspool.tile(
                                    [128, 512], f32, tag="ps", name=f"ps{i}_{j}"
                                )
                                for j in range(JP)
                            ]
                            for i in range(3)
                        ]
                        for c in range(3):
                            for j in range(JP):
                                for kh in range(3):
                                    for i in range(3):
                                        emit_mm(pss[i], 0, 3 * g + i, j, c, kh)
                        for i in range(3):
                            emit_drain(pss[i], 0, 3 * g + i)
                    rest = [(1, rt) for rt in range(RT)]
                else:
                    rest = [(cb, rt) for cb in range(2) for rt in range(RT)]

                for cb, rt in rest:
                    ps = [
                        pspool.tile([128, 512], f32, tag="ps", name=f"psj{j}")
                        for j in range(JP)
                    ]
                    for j in range(JP):
                        for c in range(3):
                            for kh in range(3):
                                emit_mm(ps, cb, rt, j, c, kh)
                    emit_drain(ps, cb, rt)

            # ---------------- phase 2: global mean via AllReduce ------------
            sums2 = spool.tile([128, 2], f32, tag="sums2")
            nc.vector.tensor_reduce(
                sums2[:],
                sums[:].rearrange("p (c m) -> p c m", c=2),
                axis=mybir.AxisListType.X,
                op=ADD,
            )
            neg_mean = spool.tile([128, 2], f32, tag="negmean")
            if n_cores > 1:
                cc_in = drampool.tile([128, 2], f32)
                cc_out = drampool.tile([128, 2], f32)
                nc.sync.dma_start(cc_in[:], sums2[:])
                nc.gpsimd.collective_compute(
                    "AllReduce",
                    ADD,
                    replica_groups=[list(range(n_cores))],
                    ins=[cc_in.opt()],
                    outs=[cc_out.opt()],
                )
                sums_g = spool.tile([128, 2], f32, tag="sumsg")
                nc.sync.dma_start(sums_g[:], cc_out[:])
                src_sums = sums_g
            else:
                src_sums = sums2
            nc.vector.tensor_scalar(
                neg_mean[:], src_sums[:], -MEAN_SCALE, 0.0, MUL, ADD
            )

            # ---------------- phase 3: binarize + store ---------------------
            # DVE takes 6 blocks (incl. the last, per-tile), GpSimd 2.
            IS_GE = mybir.AluOpType.is_ge
            for b in range(2 * N_PER_CORE):
                cb, n = divmod(b, N_PER_CORE)
                t0 = b * RT
                nm = neg_mean[:, cb : cb + 1]
                if b == 2 * N_PER_CORE - 1:
                    for i in range(RT):
                        t = t0 + i
                        bt = btpool.tile([128, 2 * TFREE], fp8, tag="bint")
                        nc.vector.tensor_scalar(
                            bt[:],
                            y_sb[:, 2 * t * TFREE : (2 * t + 2) * TFREE],
                            nm, 0.0, ADD, IS_GE,
                        )
                        nc.sync.dma_start(
                            y_d[n, cb][:, i * 2 * TFREE : (i + 1) * 2 * TFREE],
                            bt[:].bitcast(mybir.dt.uint8),
                        )
                else:
                    e = nc.gpsimd if b in (0, 2) else nc.vector
                    bin_t = opool.tile([128, RT * 2 * TFREE], fp8, tag="bin")
                    e.tensor_scalar(
                        bin_t[:],
                        y_sb[:, 2 * t0 * TFREE : 2 * (t0 + RT) * TFREE],
                        nm, 0.0, ADD, IS_GE,
                    )
                    nc.sync.dma_start(y_d[n, cb], bin_t[:].bitcast(mybir.dt.uint8))

    nc.compile()
    return nc


def prep_inputs(x, weight, bias):
    """Host-side shard + Winograd transform + fp8 split."""
    assert x.shape == (N_TOT, CI, H, W) and x.dtype == np.float32

    xs = np.ascontiguousarray(
        x.reshape(N_CORES, N_PER_CORE, 2, 128, H, W).transpose(0, 1, 3, 2, 4, 5)
    )  # [core, n, ci_f, ci_b, 56, 56]
    a = xs[..., 0:54:2]
    b = xs[..., 1:55:2]
    c = xs[..., 2:56:2]
    d = xs[..., 3:56:2]
    dt = np.stack([a - c, b + c, c - b, b - d], axis=4)  # [.., ci_b, j, 56h, 27]

    c1 = dt.astype(FP8)
    r1 = dt - c1.astype(np.float32)
    c2 = (r1 * np.float32(C_SCALE)).astype(FP8)
    r2 = r1 - c2.astype(np.float32) * np.float32(1.0 / C_SCALE)
    c3 = (r2 * np.float32(C_SCALE)).astype(FP8)

    def halves(cq):
        # [core, n, 128, 2, 4, 56, 27] -> head rows 0-28, tail rows 27-55
        hd = cq[..., 0:HROWS, :].reshape(N_CORES, N_PER_CORE, 128, 2, JP, HROWS * NP_)
        tl = cq[..., TAIL_R0:, :].reshape(N_CORES, N_PER_CORE, 128, 2, JP, HROWS * NP_)
        pad = ((0, 0),) * 5 + ((0, JPAD - HROWS * NP_),)
        shp = (N_CORES, N_PER_CORE, 128, 2, JP * JPAD)
        return np.pad(hd, pad).reshape(shp), np.pad(tl, pad).reshape(shp)

    hs, ts = zip(*(halves(q) for q in (c1, c2, c3)))

    wb = np.where(weight >= 0, np.float32(1.0), np.float32(-1.0))
    g0 = wb[:, :, :, 0]
    g1 = wb[:, :, :, 1]
    g2 = wb[:, :, :, 2]
    gt = np.stack(
        [g0, (g0 + g1 + g2) / 2, (g0 - g1 + g2) / 2, g2], axis=3
    ).astype(np.float32)  # [co, ci, kh, j]
    # [co_b, co_f, ci_b, ci_f, kh, j] -> [ci_f, co_b, j, kh, ci_b, co_f]
    g6 = gt.reshape(2, 128, 2, 128, 3, JP)
    wt = np.ascontiguousarray(g6.transpose(3, 0, 5, 4, 2, 1))
    w1 = wt.astype(FP8)
    ws = (wt * np.float32(1.0 / C_SCALE)).astype(FP8)
    assert np.all(w1.astype(np.float32) == wt)
    assert np.all(ws.astype(np.float32) * C_SCALE == wt)

    out = []
    for core in range(N_CORES):
        m = {"w1": w1, "ws": ws}
        for ci in range(3):
            m[f"xh{ci}"] = hs[ci][core]
            m[f"xt{ci}"] = ts[ci][core]
        out.append(m)
    return out


def gather(results):
    """[{y: [4,2,128,2916] fp8 {0,1}}] * 8 -> (32, 256, 54, 54) fp32 +-1.

    Per row-tile the 486 bytes are [even 9x27 | odd 9x27]; de-interleave."""
    ys = np.stack([np.asarray(r["y"]).view(FP8) for r in results]).astype(np.float32)
    ys = ys.reshape(N_CORES, N_PER_CORE, 2, 128, RT, 2, RROWS, NP_)
    out = np.empty((N_CORES, N_PER_CORE, 2, 128, RT, RROWS, OW), np.float32)
    out[..., 0::2] = ys[:, :, :, :, :, 0]
    out[..., 1::2] = ys[:, :, :, :, :, 1]
    return out.reshape(N_TOT, CO, OH, OW) * np.float32(2.0) - np.float32(1.0)


_STATE = {}


def _get_nc():
    if "nc" not in _STATE:
        import concourse.bacc as bacc

        nc = bacc.Bacc(
            "TRN2", target_bir_lowering=False, debug=False, num_devices=N_CORES
        )
        _STATE["nc"] = build(nc)
    return _STATE["nc"]


def kernel(x, weight, bias, _trace=False):
    from concourse.bass_utils import run_bass_kernel_spmd

    nc = _get_nc()
    in_maps = prep_inputs(
        np.asarray(x, np.float32),
        np.asarray(weight, np.float32),
        np.asarray(bias, np.float32),
    )
    res = run_bass_kernel_spmd(
        nc, in_maps, core_ids=list(range(N_CORES)), trace=_trace
    )
    _STATE["last_result"] = res
    return gather(res.results)


# revision 61
# speedup vs baseline: 1.0331x; 1.0139x over previous
"""Binarized 3x3 conv + batchnorm(train) + sign, on 8 TRN2 NeuronCores.

Math: out = sign((y - mean)/sqrt(var+eps)) where y = conv(x, sign(w)) + sign(b)
and mean/var are per-channel batch stats.  Since sqrt(var+eps) > 0, the output
is exactly sign(y - mean_c): variance never needs to be computed.  The +-1
channel bias cancels in sign(y - mean), so it is dropped entirely.

Strategy (data-parallel over batch, 4 images/core):
 - 1-D Winograd F(2,3) along W (host-side input transform in fp32 during
   prep): 1.5x fewer MACs than direct conv.  d~[j] = B^T d per output-column
   pair (27 pairs), kernel g~ = G g in {+-1, +-1/2, +-3/2} (exact in e4m3),
   vertical taps stay direct (3 kh).  y_even = m0+m1+m2, y_odd = m1-m2-m3.
 - all matmuls fp8-e4m3 DoubleRow (0.5 PE cycles/output-row), 243-wide
   contiguous rhs slices (no wrap waste), 36 matmuls per output tile
   (4 j x 3 comps x 3 kh) into 4 PSUM accumulators.
 - fp32-quality via the 3-component split d~ ~= c1 + c2/64 + c3/64 with the
   /64 folded into the weights (g~, g~/64).  Measured on the reference
   inputs: 148/23.9M sign flips (rel err 5.0e-3).
 - drains combine the 4 j-accumulators into y_sb even|odd halves on
   VectorE/GpSimd, harvesting per-channel sums for the mean for free.
 - one tiny AllReduce (128x2 fp32) across the 8 cores for the global mean.
 - binarize (y + (-mean)) >= 0 on VectorE/GpSimd as fp8 {0,1} bytes, the
   last block per-tile so only one small chain trails the PE stream; host
   de-interleaves even/odd and maps to +-1 fp32.
"""

import sys

if "/opt/trn_rl_repo" not in sys.path:
    sys.path.insert(0, "/opt/trn_rl_repo")

import numpy as np
import ml_dtypes

N_CORES = 8
N_PER_CORE = 4          # images per core
CI = 256                # in channels
CO = 256                # out channels
H = W = 56
OH = OW = 54
NPIX = OH * OW          # 2916
RT = 3                  # row tiles per image (18 rows each)
RROWS = 18
NP_ = 27                # output column pairs
JP = 4                  # Winograd positions
TFREE = RROWS * NP_     # 243 outputs per tile half (even or odd)
N_TOT = N_CORES * N_PER_CORE
MEAN_SCALE = 1.0 / (N_TOT * NPIX)
C_SCALE = 64.0          # residual components stored at 64x, weights at 1/64
NT = N_PER_CORE * 2 * RT  # 48 tiles per core
# head rows 0-37 (tiles 0-1), tail rows 36-55 (tile 2); per-j planes padded
# so the ci-block pair strides stay 16B-aligned
HROWS = 38
TROWS = 20
TAIL_R0 = 36
JPAD = 1040             # head: 38*27=1026 -> 1040
JPADT = 544             # tail: 20*27=540 -> 544

# binarize engine per block (cb*4+n): v=DVE is_ge {0,1}, a=ScalarE Sign
# {-1,0,1}, g=GpSimd is_ge; block 7 runs per-tile on DVE
BIN_ENG = ["v", "a", "g", "v", "a", "g", "a", "v"]

FP8 = ml_dtypes.float8_e4m3


def build(nc, n_cores=N_CORES):
    """Emit the SPMD program into a bacc.Bacc instance."""
    import concourse.mybir as mybir
    from concourse import tile

    f32 = mybir.dt.float32
    fp8 = mybir.dt.float8e4
    DR = mybir.MatmulPerfMode.DoubleRow
    ACT = mybir.ActivationFunctionType
    ADD = mybir.AluOpType.add
    MUL = mybir.AluOpType.mult

    xh_d = [
        nc.dram_tensor(f"xh{c}", [N_PER_CORE, 128, 2, JP * JPAD], fp8, kind="ExternalInput")
        for c in range(3)
    ]
    xt_d = [
        nc.dram_tensor(f"xt{c}", [N_PER_CORE, 128, 2, JP * JPADT], fp8, kind="ExternalInput")
        for c in range(3)
    ]
    w1_d = nc.dram_tensor("w1", [128, 2, JP, 3, 2, 128], fp8, kind="ExternalInput")
    ws_d = nc.dram_tensor("ws", [128, 2, JP, 3, 2, 128], fp8, kind="ExternalInput")
    y_d = nc.dram_tensor("y", [N_PER_CORE, 2, 128, NPIX], mybir.dt.uint8, kind="ExternalOutput")

    with tile.TileContext(nc) as tc:
        with (
            tc.tile_pool(name="wpool", bufs=1) as wpool,
            tc.tile_pool(name="xpool", bufs=2) as xpool,
            tc.tile_pool(name="ypool", bufs=1) as ypool,
            tc.tile_pool(name="spool", bufs=1) as spool,
            tc.tile_pool(name="opool", bufs=2) as opool,
            tc.tile_pool(name="btpool", bufs=6) as btpool,
            tc.tile_pool(name="pspool", bufs=8, space="PSUM") as pspool,
            tc.tile_pool(name="drampool", bufs=2, space="DRAM") as drampool,
        ):
            w1_sb = wpool.tile([128, 2, JP, 3, 2, 128], fp8, tag="w1")
            ws_sb = wpool.tile([128, 2, JP, 3, 2, 128], fp8, tag="ws")
            y_sb = ypool.tile([128, NT * 2 * TFREE], f32)
            sums = spool.tile([128, 4 * NT], f32, tag="sums")

            # ---------------- phase 1: conv + drain (+sums) ------------------
            # HWDGE serializes all transfers; order startup by first use.
            # First 3-tile group runs component-major so the opening matmuls
            # need only w1[cb0] + comp-1 heads (j01 first).
            for n in range(N_PER_CORE):
                xh = [
                    xpool.tile([128, 2, JP * JPAD], fp8, tag=f"xh{c}", name=f"xh{c}")
                    for c in range(3)
                ]
                xt = [
                    xpool.tile([128, 2, JP * JPADT], fp8, tag=f"xt{c}", name=f"xt{c}")
                    for c in range(3)
                ]
                if n == 0:
                    nc.sync.dma_start(w1_sb[:, 0], w1_d[:, 0])
                    nc.scalar.dma_start(xh[0][:, :, 0 : 2 * JPAD], xh_d[0][n][:, :, 0 : 2 * JPAD])
                    nc.sync.dma_start(ws_sb[:, 0], ws_d[:, 0])
                    nc.scalar.dma_start(xh[1][:, :, 0 : 2 * JPAD], xh_d[1][n][:, :, 0 : 2 * JPAD])
                    nc.sync.dma_start(xh[0][:, :, 2 * JPAD :], xh_d[0][n][:, :, 2 * JPAD :])
                    nc.scalar.dma_start(xh[2][:, :, 0 : 2 * JPAD], xh_d[2][n][:, :, 0 : 2 * JPAD])
                    nc.sync.dma_start(xh[1][:, :, 2 * JPAD :], xh_d[1][n][:, :, 2 * JPAD :])
                    nc.scalar.dma_start(xt[0][:], xt_d[0][n])
                    nc.sync.dma_start(xh[2][:, :, 2 * JPAD :], xh_d[2][n][:, :, 2 * JPAD :])
                    nc.scalar.dma_start(xt[1][:], xt_d[1][n])
                    nc.sync.dma_start(xt[2][:], xt_d[2][n])
                    nc.sync.dma_start(w1_sb[:, 1], w1_d[:, 1])
                    nc.scalar.dma_start(ws_sb[:, 1], ws_d[:, 1])
                else:
                    nc.sync.dma_start(xh[0][:], xh_d[0][n])
                    nc.sync.dma_start(xt[0][:], xt_d[0][n])
                    nc.sync.dma_start(xh[1][:], xh_d[1][n])
                    nc.sync.dma_start(xt[1][:], xt_d[1][n])
                    nc.sync.dma_start(xh[2][:], xh_d[2][n])
                    nc.sync.dma_start(xt[2][:], xt_d[2][n])

                def emit_mm(ps_t, cb, rt, j, c, kh):
                    w_sb = w1_sb if c == 0 else ws_sb
                    row = rt * RROWS + kh
                    if rt < 2:
                        src, base, jp = xh[c], 0, JPAD
                    else:
                        src, base, jp = xt[c], TAIL_R0, JPADT
                    off = j * jp + (row - base) * NP_
                    nc.tensor.matmul(
                        ps_t[j][:, 0:TFREE],
                        w_sb[:, cb, j, kh],
                        src[:, :, off : off + TFREE],
                        start=(c == 0 and kh == 0),
                        stop=(c == 2 and kh == 2),
                        perf_mode=DR,
                    )

                def emit_drain(ps_t, cb, rt):
                    t = (cb * N_PER_CORE + n) * RT + rt
                    ev = y_sb[:, 2 * t * TFREE : (2 * t + 1) * TFREE]
                    od = y_sb[:, (2 * t + 1) * TFREE : (2 * t + 2) * TFREE]
                    p0 = ps_t[0][:, 0:TFREE]
                    p1 = ps_t[1][:, 0:TFREE]
                    p2 = ps_t[2][:, 0:TFREE]
                    p3 = ps_t[3][:, 0:TFREE]
                    # HW: only ONE PSUM operand per DVE op, and GpSimd
                    # cannot read PSUM at all.  ScalarE copies j0/j1/j2 out
                    # (harvesting their sums A,B,C), GpSimd adds the even
                    # half in SBUF, DVE adds the odd half (PSUM j3 last,
                    # harvesting O).  Sigma-y per tile = A+B+C+O.
                    s2 = btpool.tile([128, TFREE], f32, tag="s2", name="s2")
                    nc.scalar.activation(
                        ev, p0, ACT.Copy, accum_out=sums[:, 4 * t : 4 * t + 1]
                    )
                    nc.scalar.activation(
                        od, p1, ACT.Copy, accum_out=sums[:, 4 * t + 1 : 4 * t + 2]
                    )
                    nc.scalar.activation(
                        s2[:], p2, ACT.Copy, accum_out=sums[:, 4 * t + 2 : 4 * t + 3]
                    )
                    nc.vector.tensor_add(ev, ev, od)
                    nc.vector.tensor_add(ev, ev, s2[:])
                    nc.vector.tensor_sub(od, od, s2[:])
                    nc.vector.scalar_tensor_tensor(
                        od, p3, -1.0, od, MUL, ADD,
                        accum_out=sums[:, 4 * t + 3 : 4 * t + 4],
                    )

                if n == 0:
                    # cb0 runs component-major so each comp's transfers
                    # stream in just ahead of their first use
                    pss = [
                        [
                            pspool.tile(
                                [128, 512], f32, tag="ps", name=f"ps{i}_{j}"
                            )
                            for j in range(JP)
                        ]
                        for i in range(2)
                    ]
                    for c in range(3):
                        for j in range(JP):
                            for kh in range(3):
                                for i in range(2):
                                    emit_mm(pss[i], 0, i, j, c, kh)
                    for i in range(2):
                        emit_drain(pss[i], 0, i)
                    rest = [(0, 2)] + [(1, rt) for rt in range(RT)]
                else:
                    rest = [(cb, rt) for cb in range(2) for rt in range(RT)]

                for cb, rt in rest:
                    ps = [
                        pspool.tile([128, 512], f32, tag="ps", name=f"psj{j}")
                        for j in range(JP)
                    ]
                    for j in range(JP):
                        for c in range(3):
                            for kh in range(3):
                                emit_mm(ps, cb, rt, j, c, kh)
                    emit_drain(ps, cb, rt)

            # ---------------- phase 2: global mean via AllReduce ------------
            sums2 = spool.tile([128, 2], f32, tag="sums2")
            nc.vector.tensor_reduce(
                sums2[:],
                sums[:].rearrange("p (c m) -> p c m", c=2),
                axis=mybir.AxisListType.X,
                op=ADD,
            )
            neg_mean = spool.tile([128, 2], f32, tag="negmean")
            if n_cores > 1:
                cc_in = drampool.tile([128, 2], f32)
                cc_out = drampool.tile([128, 2], f32)
                nc.sync.dma_start(cc_in[:], sums2[:])
                nc.gpsimd.collective_compute(
                    "AllReduce",
                    ADD,
                    replica_groups=[list(range(n_cores))],
                    ins=[cc_in.opt()],
                    outs=[cc_out.opt()],
                )
                sums_g = spool.tile([128, 2], f32, tag="sumsg")
                nc.sync.dma_start(sums_g[:], cc_out[:])
                src_sums = sums_g
            else:
                src_sums = sums2
            nc.vector.tensor_scalar(
                neg_mean[:], src_sums[:], -MEAN_SCALE, 0.0, MUL, ADD
            )

            # ---------------- phase 3: binarize + store ---------------------
            # blocks spread over DVE / ScalarE(Sign) / GpSimd per BIN_ENG;
            # the last block runs per-tile on DVE.
            IS_GE = mybir.AluOpType.is_ge
            for b in range(2 * N_PER_CORE):
                cb, n = divmod(b, N_PER_CORE)
                t0 = b * RT
                nm = neg_mean[:, cb : cb + 1]
                if b == 2 * N_PER_CORE - 1:
                    for i in range(RT):
                        t = t0 + i
                        bt = btpool.tile([128, 2 * TFREE], fp8, tag="bint")
                        nc.vector.tensor_scalar(
                            bt[:],
                            y_sb[:, 2 * t * TFREE : (2 * t + 2) * TFREE],
                            nm, 0.0, ADD, IS_GE,
                        )
                        nc.sync.dma_start(
                            y_d[n, cb][:, i * 2 * TFREE : (i + 1) * 2 * TFREE],
                            bt[:].bitcast(mybir.dt.uint8),
                        )
                else:
                    e = nc.gpsimd if b in (0, 2) else nc.vector
                    bin_t = opool.tile([128, RT * 2 * TFREE], fp8, tag="bin")
                    e.tensor_scalar(
                        bin_t[:],
                        y_sb[:, 2 * t0 * TFREE : 2 * (t0 + RT) * TFREE],
                        nm, 0.0, ADD, IS_GE,
                    )
                    nc.sync.dma_start(y_d[n, cb], bin_t[:].bitcast(mybir.dt.uint8))

    nc.compile()
    return nc


def prep_inputs(x, weight, bias):
    """Host-side shard + Winograd transform + fp8 split."""
    assert x.shape == (N_TOT, CI, H, W) and x.dtype == np.float32

    xs = np.ascontiguousarray(
        x.reshape(N_CORES, N_PER_CORE, 2, 128, H, W).transpose(0, 1, 3, 2, 4, 5)
    )  # [core, n, ci_f, ci_b, 56, 56]
    a = xs[..., 0:54:2]
    b = xs[..., 1:55:2]
    c = xs[..., 2:56:2]
    d = xs[..., 3:56:2]
    dt = np.stack([a - c, b + c, c - b, b - d], axis=4)  # [.., ci_b, j, 56h, 27]

    c1 = dt.astype(FP8)
    r1 = dt - c1.astype(np.float32)
    c2 = (r1 * np.float32(C_SCALE)).astype(FP8)
    r2 = r1 - c2.astype(np.float32) * np.float32(1.0 / C_SCALE)
    c3 = (r2 * np.float32(C_SCALE)).astype(FP8)

    def halves(cq):
        # [core, n, 128, 2, 4, 56, 27] -> head rows 0-37, tail rows 36-55
        hd = cq[..., 0:HROWS, :].reshape(N_CORES, N_PER_CORE, 128, 2, JP, HROWS * NP_)
        tl = cq[..., TAIL_R0:, :].reshape(N_CORES, N_PER_CORE, 128, 2, JP, TROWS * NP_)
        hd = np.pad(hd, ((0, 0),) * 5 + ((0, JPAD - HROWS * NP_),))
        tl = np.pad(tl, ((0, 0),) * 5 + ((0, JPADT - TROWS * NP_),))
        return (
            hd.reshape(N_CORES, N_PER_CORE, 128, 2, JP * JPAD),
            tl.reshape(N_CORES, N_PER_CORE, 128, 2, JP * JPADT),
        )

    hs, ts = zip(*(halves(q) for q in (c1, c2, c3)))

    wb = np.where(weight >= 0, np.float32(1.0), np.float32(-1.0))
    g0 = wb[:, :, :, 0]
    g1 = wb[:, :, :, 1]
    g2 = wb[:, :, :, 2]
    gt = np.stack(
        [g0, (g0 + g1 + g2) / 2, (g0 - g1 + g2) / 2, g2], axis=3
    ).astype(np.float32)  # [co, ci, kh, j]
    # [co_b, co_f, ci_b, ci_f, kh, j] -> [ci_f, co_b, j, kh, ci_b, co_f]
    g6 = gt.reshape(2, 128, 2, 128, 3, JP)
    wt = np.ascontiguousarray(g6.transpose(3, 0, 5, 4, 2, 1))
    w1 = wt.astype(FP8)
    ws = (wt * np.float32(1.0 / C_SCALE)).astype(FP8)
    assert np.all(w1.astype(np.float32) == wt)
    assert np.all(ws.astype(np.float32) * C_SCALE == wt)

    out = []
    for core in range(N_CORES):
        m = {"w1": w1, "ws": ws}
        for ci in range(3):
            m[f"xh{ci}"] = hs[ci][core]
            m[f"xt{ci}"] = ts[ci][core]
        out.append(m)
    return out


def gather(results):
    """[{y: [4,2,128,2916] fp8 {0,1}}] * 8 -> (32, 256, 54, 54) fp32 +-1.

    Per row-tile the 486 bytes are [even 9x27 | odd 9x27]; de-interleave."""
    ys = np.stack([np.asarray(r["y"]).view(FP8) for r in results]).astype(np.float32)
    ys = ys.reshape(N_CORES, N_PER_CORE, 2, 128, RT, 2, RROWS, NP_)
    out = np.empty((N_CORES, N_PER_CORE, 2, 128, RT, RROWS, OW), np.float32)
    out[..., 0::2] = ys[:, :, :, :, :, 0]
    out[..., 1::2] = ys[:, :, :, :, :, 1]
    return out.reshape(N_TOT, CO, OH, OW) * np.float32(2.0) - np.float32(1.0)


_STATE = {}


def _get_nc():
    if "nc" not in _STATE:
        import concourse.bacc as bacc

        nc = bacc.Bacc(
            "TRN2", target_bir_lowering=False, debug=False, num_devices=N_CORES
        )
        _STATE["nc"] = build(nc)
    return _STATE["nc"]


def kernel(x, weight, bias, _trace=False):
    from concourse.bass_utils import run_bass_kernel_spmd

    nc = _get_nc()
    in_maps = prep_inputs(
        np.asarray(x, np.float32),
        np.asarray(weight, np.float32),
        np.asarray(bias, np.float32),
    )
    res = run_bass_kernel_spmd(
        nc, in_maps, core_ids=list(range(N_CORES)), trace=_trace
    )
    _STATE["last_result"] = res
    return gather(res.results)


# revision 63
# speedup vs baseline: 1.0898x; 1.0550x over previous
"""Binarized 3x3 conv + batchnorm(train) + sign, on 8 TRN2 NeuronCores.

Math: out = sign((y - mean)/sqrt(var+eps)) where y = conv(x, sign(w)) + sign(b)
and mean/var are per-channel batch stats.  Since sqrt(var+eps) > 0, the output
is exactly sign(y - mean_c): variance never needs to be computed.  The +-1
channel bias cancels in sign(y - mean), so it is dropped entirely.

Strategy (data-parallel over batch, 4 images/core):
 - 1-D Winograd F(2,3) along W (host-side input transform in fp32 during
   prep): 1.5x fewer MACs than direct conv.  d~[j] = B^T d per output-column
   pair (27 pairs), kernel g~ = G g in {+-1, +-1/2, +-3/2} (exact in e4m3),
   vertical taps stay direct (3 kh).  y_even = m0+m1+m2, y_odd = m1-m2-m3.
 - all matmuls fp8-e4m3 DoubleRow (0.5 PE cycles/output-row), 243-wide
   contiguous rhs slices (no wrap waste), 36 matmuls per output tile
   (4 j x 3 comps x 3 kh) into 4 PSUM accumulators.
 - fp32-quality via the 3-component split d~ ~= c1 + c2/64 + c3/64 with the
   /64 folded into the weights (g~, g~/64).  Measured on the reference
   inputs: 148/23.9M sign flips (rel err 5.0e-3).
 - drains combine the 4 j-accumulators into y_sb even|odd halves on
   VectorE/GpSimd, harvesting per-channel sums for the mean for free.
 - one tiny AllReduce (128x2 fp32) across the 8 cores for the global mean.
 - binarize (y + (-mean)) >= 0 on VectorE/GpSimd as fp8 {0,1} bytes, the
   last block per-tile so only one small chain trails the PE stream; host
   de-interleaves even/odd and maps to +-1 fp32.
"""

import sys

if "/opt/trn_rl_repo" not in sys.path:
    sys.path.insert(0, "/opt/trn_rl_repo")

import numpy as np
import ml_dtypes

N_CORES = 8
N_PER_CORE = 4          # images per core
CI = 256                # in channels
CO = 256                # out channels
H = W = 56
OH = OW = 54
NPIX = OH * OW          # 2916
RT = 3                  # row tiles per image (18 rows each)
RROWS = 18
NP_ = 27                # output column pairs
JP = 4                  # Winograd positions
TFREE = RROWS * NP_     # 243 outputs per tile half (even or odd)
N_TOT = N_CORES * N_PER_CORE
MEAN_SCALE = 1.0 / (N_TOT * NPIX)
C_SCALE = 64.0          # residual components stored at 64x, weights at 1/64
NT = N_PER_CORE * 2 * RT  # 48 tiles per core
# head rows 0-37 (tiles 0-1), tail rows 36-55 (tile 2); per-j planes padded
# so the ci-block pair strides stay 16B-aligned
HROWS = 38
TROWS = 20
TAIL_R0 = 36
JPAD = 1040             # head: 38*27=1026 -> 1040
JPADT = 544             # tail: 20*27=540 -> 544

# binarize engine per block (cb*4+n): v=DVE is_ge {0,1}, a=ScalarE Sign
# {-1,0,1}, g=GpSimd is_ge; block 7 runs per-tile on DVE
BIN_ENG = ["v", "a", "g", "v", "a", "g", "a", "v"]

FP8 = ml_dtypes.float8_e4m3


def build(nc, n_cores=N_CORES):
    """Emit the SPMD program into a bacc.Bacc instance."""
    import concourse.mybir as mybir
    from concourse import tile

    f32 = mybir.dt.float32
    fp8 = mybir.dt.float8e4
    DR = mybir.MatmulPerfMode.DoubleRow
    ACT = mybir.ActivationFunctionType
    ADD = mybir.AluOpType.add
    MUL = mybir.AluOpType.mult

    xh_d = [
        nc.dram_tensor(f"xh{c}", [N_PER_CORE, 128, 2, JP * JPAD], fp8, kind="ExternalInput")
        for c in range(3)
    ]
    xt_d = [
        nc.dram_tensor(f"xt{c}", [N_PER_CORE, 128, 2, JP * JPADT], fp8, kind="ExternalInput")
        for c in range(3)
    ]
    w1_d = nc.dram_tensor("w1", [128, 2, JP, 3, 2, 128], fp8, kind="ExternalInput")
    ws_d = nc.dram_tensor("ws", [128, 2, JP, 3, 2, 128], fp8, kind="ExternalInput")
    y_d = nc.dram_tensor("y", [N_PER_CORE, 2, 128, NPIX], mybir.dt.uint8, kind="ExternalOutput")

    with tile.TileContext(nc) as tc:
        with (
            tc.tile_pool(name="wpool", bufs=1) as wpool,
            tc.tile_pool(name="xpool", bufs=2) as xpool,
            tc.tile_pool(name="ypool", bufs=1) as ypool,
            tc.tile_pool(name="spool", bufs=1) as spool,
            tc.tile_pool(name="opool", bufs=2) as opool,
            tc.tile_pool(name="btpool", bufs=6) as btpool,
            tc.tile_pool(name="pspool", bufs=8, space="PSUM") as pspool,
            tc.tile_pool(name="drampool", bufs=2, space="DRAM") as drampool,
        ):
            w1_sb = wpool.tile([128, 2, JP, 3, 2, 128], fp8, tag="w1")
            ws_sb = wpool.tile([128, 2, JP, 3, 2, 128], fp8, tag="ws")
            y_sb = ypool.tile([128, NT * 2 * TFREE], f32)
            sums = spool.tile([128, 4 * NT], f32, tag="sums")
            sums2 = spool.tile([128, 2], f32, tag="sums2")
            neg_mean = spool.tile([128, 2], f32, tag="negmean")

            def emit_mean_cb(cb):
                # cb's per-channel mean: reduce its 48 sum cols + scale.
                # (In the 8-core path the AllReduce for this half follows in
                # phase 2; the timed single-core path uses sums2 directly.)
                nc.vector.tensor_reduce(
                    sums2[:, cb : cb + 1],
                    sums[:, cb * 48 : (cb + 1) * 48].rearrange(
                        "p (a m) -> p a m", a=1
                    ),
                    axis=mybir.AxisListType.X,
                    op=ADD,
                )
                if n_cores == 1:
                    nc.vector.tensor_scalar(
                        neg_mean[:, cb : cb + 1],
                        sums2[:, cb : cb + 1],
                        -MEAN_SCALE, 0.0, MUL, ADD,
                    )

            # ---------------- phase 1: conv + drain (+sums) ------------------
            # HWDGE serializes all transfers; order startup by first use.
            # First 3-tile group runs component-major so the opening matmuls
            # need only w1[cb0] + comp-1 heads (j01 first).
            for n in range(N_PER_CORE):
                xh = [
                    xpool.tile([128, 2, JP * JPAD], fp8, tag=f"xh{c}", name=f"xh{c}")
                    for c in range(3)
                ]
                xt = [
                    xpool.tile([128, 2, JP * JPADT], fp8, tag=f"xt{c}", name=f"xt{c}")
                    for c in range(3)
                ]
                if n == 0:
                    nc.sync.dma_start(w1_sb[:, 0], w1_d[:, 0])
                    nc.scalar.dma_start(xh[0][:, :, 0 : 2 * JPAD], xh_d[0][n][:, :, 0 : 2 * JPAD])
                    nc.sync.dma_start(ws_sb[:, 0], ws_d[:, 0])
                    nc.scalar.dma_start(xh[1][:, :, 0 : 2 * JPAD], xh_d[1][n][:, :, 0 : 2 * JPAD])
                    nc.sync.dma_start(xh[0][:, :, 2 * JPAD :], xh_d[0][n][:, :, 2 * JPAD :])
                    nc.scalar.dma_start(xh[2][:, :, 0 : 2 * JPAD], xh_d[2][n][:, :, 0 : 2 * JPAD])
                    nc.sync.dma_start(xh[1][:, :, 2 * JPAD :], xh_d[1][n][:, :, 2 * JPAD :])
                    nc.scalar.dma_start(xt[0][:], xt_d[0][n])
                    nc.sync.dma_start(xh[2][:, :, 2 * JPAD :], xh_d[2][n][:, :, 2 * JPAD :])
                    nc.scalar.dma_start(xt[1][:], xt_d[1][n])
                    nc.sync.dma_start(xt[2][:], xt_d[2][n])
                    nc.sync.dma_start(w1_sb[:, 1], w1_d[:, 1])
                    nc.scalar.dma_start(ws_sb[:, 1], ws_d[:, 1])
                else:
                    nc.sync.dma_start(xh[0][:], xh_d[0][n])
                    nc.sync.dma_start(xt[0][:], xt_d[0][n])
                    nc.sync.dma_start(xh[1][:], xh_d[1][n])
                    nc.sync.dma_start(xt[1][:], xt_d[1][n])
                    nc.sync.dma_start(xh[2][:], xh_d[2][n])
                    nc.sync.dma_start(xt[2][:], xt_d[2][n])

                def emit_mm(ps_t, cb, rt, j, c, kh):
                    w_sb = w1_sb if c == 0 else ws_sb
                    row = rt * RROWS + kh
                    if rt < 2:
                        src, base, jp = xh[c], 0, JPAD
                    else:
                        src, base, jp = xt[c], TAIL_R0, JPADT
                    off = j * jp + (row - base) * NP_
                    nc.tensor.matmul(
                        ps_t[j][:, 0:TFREE],
                        w_sb[:, cb, j, kh],
                        src[:, :, off : off + TFREE],
                        start=(c == 0 and kh == 0),
                        stop=(c == 2 and kh == 2),
                        perf_mode=DR,
                    )

                def emit_drain(ps_t, cb, rt):
                    t = (cb * N_PER_CORE + n) * RT + rt
                    ev = y_sb[:, 2 * t * TFREE : (2 * t + 1) * TFREE]
                    od = y_sb[:, (2 * t + 1) * TFREE : (2 * t + 2) * TFREE]
                    p0 = ps_t[0][:, 0:TFREE]
                    p1 = ps_t[1][:, 0:TFREE]
                    p2 = ps_t[2][:, 0:TFREE]
                    p3 = ps_t[3][:, 0:TFREE]
                    # HW: only ONE PSUM operand per DVE op, and GpSimd
                    # cannot read PSUM at all.  ScalarE copies j0/j1/j2 out
                    # (harvesting their sums A,B,C), GpSimd adds the even
                    # half in SBUF, DVE adds the odd half (PSUM j3 last,
                    # harvesting O).  Sigma-y per tile = A+B+C+O.
                    s2 = btpool.tile([128, TFREE], f32, tag="s2", name="s2")
                    nc.scalar.activation(
                        ev, p0, ACT.Copy, accum_out=sums[:, 4 * t : 4 * t + 1]
                    )
                    nc.scalar.activation(
                        od, p1, ACT.Copy, accum_out=sums[:, 4 * t + 1 : 4 * t + 2]
                    )
                    nc.scalar.activation(
                        s2[:], p2, ACT.Copy, accum_out=sums[:, 4 * t + 2 : 4 * t + 3]
                    )
                    nc.vector.tensor_add(ev, ev, od)
                    nc.vector.tensor_add(ev, ev, s2[:])
                    nc.vector.tensor_sub(od, od, s2[:])
                    nc.vector.scalar_tensor_tensor(
                        od, p3, -1.0, od, MUL, ADD,
                        accum_out=sums[:, 4 * t + 3 : 4 * t + 4],
                    )

                if n == 0:
                    # cb0 runs component-major so each comp's transfers
                    # stream in just ahead of their first use
                    pss = [
                        [
                            pspool.tile(
                                [128, 512], f32, tag="ps", name=f"ps{i}_{j}"
                            )
                            for j in range(JP)
                        ]
                        for i in range(2)
                    ]
                    for c in range(3):
                        for j in range(JP):
                            for kh in range(3):
                                for i in range(2):
                                    emit_mm(pss[i], 0, i, j, c, kh)
                    for i in range(2):
                        emit_drain(pss[i], 0, i)
                    rest = [(0, 2)] + [(1, rt) for rt in range(RT)]
                else:
                    rest = [(cb, rt) for cb in range(2) for rt in range(RT)]

                for cb, rt in rest:
                    ps = [
                        pspool.tile([128, 512], f32, tag="ps", name=f"psj{j}")
                        for j in range(JP)
                    ]
                    for j in range(JP):
                        for c in range(3):
                            for kh in range(3):
                                emit_mm(ps, cb, rt, j, c, kh)
                    emit_drain(ps, cb, rt)
                    if n == N_PER_CORE - 1 and cb == 0 and rt == RT - 1:
                        # all cb0 sums are in: compute its mean now (DVE is
                        # idle-waiting on cb1 stops here) so GpSimd's cb0
                        # binarize blocks overlap the remaining PE work
                        emit_mean_cb(0)

            # ---------------- phase 2: global mean via AllReduce ------------
            # per-co-block: cb0's sums are complete ~10us before cb1's, so
            # cb0's mean (reduced on the queue-clear GpSimd) unblocks its
            # binarize blocks while the PE still computes cb1.
            emit_mean_cb(1)
            if n_cores > 1:
                for cb in range(2):
                    cc_in = drampool.tile([128, 1], f32, name=f"ccin{cb}")
                    cc_out = drampool.tile([128, 1], f32, name=f"ccout{cb}")
                    nc.sync.dma_start(cc_in[:], sums2[:, cb : cb + 1])
                    nc.gpsimd.collective_compute(
                        "AllReduce",
                        ADD,
                        replica_groups=[list(range(n_cores))],
                        ins=[cc_in.opt()],
                        outs=[cc_out.opt()],
                    )
                    sums_g = spool.tile([128, 1], f32, tag=f"sumsg{cb}")
                    nc.sync.dma_start(sums_g[:], cc_out[:])
                    nc.vector.tensor_scalar(
                        neg_mean[:, cb : cb + 1], sums_g[:],
                        -MEAN_SCALE, 0.0, MUL, ADD,
                    )

            # ---------------- phase 3: binarize + store ---------------------
            # blocks spread over DVE / ScalarE(Sign) / GpSimd per BIN_ENG;
            # the last block runs per-tile on DVE.
            IS_GE = mybir.AluOpType.is_ge
            for b in range(2 * N_PER_CORE):
                cb, n = divmod(b, N_PER_CORE)
                t0 = b * RT
                nm = neg_mean[:, cb : cb + 1]
                if b == 2 * N_PER_CORE - 1:
                    for i in range(RT):
                        t = t0 + i
                        bt = btpool.tile([128, 2 * TFREE], fp8, tag="bint")
                        nc.vector.tensor_scalar(
                            bt[:],
                            y_sb[:, 2 * t * TFREE : (2 * t + 2) * TFREE],
                            nm, 0.0, ADD, IS_GE,
                        )
                        nc.sync.dma_start(
                            y_d[n, cb][:, i * 2 * TFREE : (i + 1) * 2 * TFREE],
                            bt[:].bitcast(mybir.dt.uint8),
                        )
                else:
                    e = nc.gpsimd if cb == 0 else nc.vector
                    bin_t = opool.tile([128, RT * 2 * TFREE], fp8, tag="bin")
                    e.tensor_scalar(
                        bin_t[:],
                        y_sb[:, 2 * t0 * TFREE : 2 * (t0 + RT) * TFREE],
                        nm, 0.0, ADD, IS_GE,
                    )
                    nc.sync.dma_start(y_d[n, cb], bin_t[:].bitcast(mybir.dt.uint8))

    nc.compile()
    return nc


def prep_inputs(x, weight, bias):
    """Host-side shard + Winograd transform + fp8 split."""
    assert x.shape == (N_TOT, CI, H, W) and x.dtype == np.float32

    xs = np.ascontiguousarray(
        x.reshape(N_CORES, N_PER_CORE, 2, 128, H, W).transpose(0, 1, 3, 2, 4, 5)
    )  # [core, n, ci_f, ci_b, 56, 56]
    a = xs[..., 0:54:2]
    b = xs[..., 1:55:2]
    c = xs[..., 2:56:2]
    d = xs[..., 3:56:2]
    dt = np.stack([a - c, b + c, c - b, b - d], axis=4)  # [.., ci_b, j, 56h, 27]

    c1 = dt.astype(FP8)
    r1 = dt - c1.astype(np.float32)
    c2 = (r1 * np.float32(C_SCALE)).astype(FP8)
    r2 = r1 - c2.astype(np.float32) * np.float32(1.0 / C_SCALE)
    c3 = (r2 * np.float32(C_SCALE)).astype(FP8)

    def halves(cq):
        # [core, n, 128, 2, 4, 56, 27] -> head rows 0-37, tail rows 36-55
        hd = cq[..., 0:HROWS, :].reshape(N_CORES, N_PER_CORE, 128, 2, JP, HROWS * NP_)
        tl = cq[..., TAIL_R0:, :].reshape(N_CORES, N_PER_CORE, 128, 2, JP, TROWS * NP_)
        hd = np.pad(hd, ((0, 0),) * 5 + ((0, JPAD - HROWS * NP_),))
        tl = np.pad(tl, ((0, 0),) * 5 + ((0, JPADT - TROWS * NP_),))
        return (
            hd.reshape(N_CORES, N_PER_CORE, 128, 2, JP * JPAD),
            tl.reshape(N_CORES, N_PER_CORE, 128, 2, JP * JPADT),
        )

    hs, ts = zip(*(halves(q) for q in (c1, c2, c3)))

    wb = np.where(weight >= 0, np.float32(1.0), np.float32(-1.0))
    g0 = wb[:, :, :, 0]
    g1 = wb[:, :, :, 1]
    g2 = wb[:, :, :, 2]
    gt = np.stack(
        [g0, (g0 + g1 + g2) / 2, (g0 - g1 + g2) / 2, g2], axis=3
    ).astype(np.float32)  # [co, ci, kh, j]
    # [co_b, co_f, ci_b, ci_f, kh, j] -> [ci_f, co_b, j, kh, ci_b, co_f]
    g6 = gt.reshape(2, 128, 2, 128, 3, JP)
    wt = np.ascontiguousarray(g6.transpose(3, 0, 5, 4, 2, 1))
    w1 = wt.astype(FP8)
    ws = (wt * np.float32(1.0 / C_SCALE)).astype(FP8)
    assert np.all(w1.astype(np.float32) == wt)
    assert np.all(ws.astype(np.float32) * C_SCALE == wt)

    out = []
    for core in range(N_CORES):
        m = {"w1": w1, "ws": ws}
        for ci in range(3):
            m[f"xh{ci}"] = hs[ci][core]
            m[f"xt{ci}"] = ts[ci][core]
        out.append(m)
    return out


def gather(results):
    """[{y: [4,2,128,2916] fp8 {0,1}}] * 8 -> (32, 256, 54, 54) fp32 +-1.

    Per row-tile the 486 bytes are [even 9x27 | odd 9x27]; de-interleave."""
    ys = np.stack([np.asarray(r["y"]).view(FP8) for r in results]).astype(np.float32)
    ys = ys.reshape(N_CORES, N_PER_CORE, 2, 128, RT, 2, RROWS, NP_)
    out = np.empty((N_CORES, N_PER_CORE, 2, 128, RT, RROWS, OW), np.float32)
    out[..., 0::2] = ys[:, :, :, :, :, 0]
    out[..., 1::2] = ys[:, :, :, :, :, 1]
    return out.reshape(N_TOT, CO, OH, OW) * np.float32(2.0) - np.float32(1.0)


_STATE = {}


def _get_nc():
    if "nc" not in _STATE:
        import concourse.bacc as bacc

        nc = bacc.Bacc(
            "TRN2", target_bir_lowering=False, debug=False, num_devices=N_CORES
        )
        _STATE["nc"] = build(nc)
    return _STATE["nc"]


def kernel(x, weight, bias, _trace=False):
    from concourse.bass_utils import run_bass_kernel_spmd

    nc = _get_nc()
    in_maps = prep_inputs(
        np.asarray(x, np.float32),
        np.asarray(weight, np.float32),
        np.asarray(bias, np.float32),
    )
    res = run_bass_kernel_spmd(
        nc, in_maps, core_ids=list(range(N_CORES)), trace=_trace
    )
    _STATE["last_result"] = res
    return gather(res.results)
